# revision 20
# baseline (speedup 1.0000x reference)
"""Multi-head attention (B=1, L=4096, C=512, H=8, D=64) on 8 TRN2 NeuronCores.

Sharding: head-parallel - core h computes head h end-to-end (QKV projection
for its head, softmax attention, its partial out-projection). Host sums the
8 partial projections and adds the bias.

Default path is "v3" (this session's winner; v1/v2 kept behind flags):
  * stage 1: psum [q;k] = [wq|wk].T @ xT-slices, PSUM->SBUF copies
    alternating ScalarE/DVE; crossed SBUF->SBUF DMA builds kqT so paired
    score matmuls have aligned base partitions.  wq pre-scaled by
    D^-1/2*log2(e) on the host (scores arrive as t = s*log2e).
  * stage 2: v[L,D] bf16 + ones column (PV then accumulates softmax
    denominators for free in accumulator row D); 16 key tiles' v matmuls
    batched per PSUM tile, ONE strided DVE eviction per half.
  * attention per 512-wide query slice: 3-key-tile score GROUPS in
    [128,1536] 3-bank PSUM tiles (st_bufs=2), ONE ScalarE exp instr per
    group (amortizes the ~222cy access-latency init over 1536 cols), pv
    emitted per single key tile with a 2-group lookahead (PE stream:
    scores(g), pv-triplet(g-2)) so the in-order PE queue never head-of-line
    blocks on an in-flight exp.  Out-proj of slice i-1 spread ONE tile per
    op group {3,5,7,9} of slice i (op_bufs=1: spreading avoids the
    pp->yt->pp serialization stall of emitting 4 tiles back-to-back).
  * slice tail: row sums transposed [1,512]->[128,4] by 4 tiny PE
    transpose matmuls into an op-pool PSUM tile, then the 1-pass DVE
    reciprocal seed op writes rec_all directly (rec_mode="pe"; the old
    DRAM-bounce transpose put 2 DMA round trips on the yt critical path).
  * PSUM budget (8 banks): 2 score groups (3 banks each) + 1 PV + 1 op.
  * Measured-on-HW notes (axon wall-deltas are +/-15%; within-batch
    comparisons only): the custom 2-op DVE exp (OPA/OPB below) is
    throughput-priced as modeled (~1.37us per 512-col chunk) but ANY
    mixing of it into the exp stream measured slower end-to-end (+7us at
    2 chunks/slice, +21us all-DVE) - the PV dependency chain pays its
    latency and the kernel sits at a PE/ScalarE equilibrium (~16us/slice
    effective, consistent with partially-exposed LoadStationary, "2.1GHz
    effective PE").  fp8 DoubleRow (2x PE) is numerically DEAD here:
    e4m3 on v alone costs 1.7e-2 absmax rel err (budget 2e-2), scores
    5.8e-2.  ScalarE Exp and Copy share an act table (no reload thrash).
  * unroll=2 (two passes per For_i iteration) measured -11us/pass within
    batch: the hardware loop boundary costs ~20us/iteration.  unroll=4/8
    measured +16us WORSE than unroll=2 (larger loop bodies appear to
    thrash sequencer instruction fetch) - 2 is the sweet spot.
"""

import numpy as np
import ml_dtypes

L, C, D, H = 4096, 512, 64, 8
N_CORES = 8
P = 128

_BF16 = ml_dtypes.bfloat16

# ---- custom DVE exp: constants -------------------------------------------
MAGIC = 12582527.0          # 2^23 + 512k + 127: magic round-to-int addend
POLY_B = 2.9504             # p(f) = (f + B)*f + C  ~  K * 2^f  on [-.5, .5]
POLY_C = 4.19605
POLY_K = 4.194189908867873
A_SCALE = 128.0
A_BIAS = (MAGIC - 127.0) * 128.0
LOG2E = 1.4426950408889634
LN2 = 0.6931471805599453
LNK = float(np.log(POLY_K))

# reciprocal seed constants (from concourse.dve_ops.RECIP_APPROX_FAST_CONSTS)
RECIP_S0, RECIP_S1, RECIP_IMM2 = -0.23549792, 2.0017324, 2.0

_ops_registered = {}


def _register_dve_ops():
    """Register the two custom DVE exp micro-ops (runtime registration: the
    uop table is generated per-NEFF from dve_ops.OPS at compile time)."""
    if _ops_registered:
        return _ops_registered
    from concourse.dve_spec import Spec, Src0, Src1, C0, C1, C2, lower, _has_src1
    from concourse.dve_uop import DveOpSpec
    import concourse.dve_ops as dve_ops
    from concourse.dve_ops import DveOp

    def _refA(in0, in1, c0, c1, c2):
        z = (in0.astype(np.float32) + np.float32(c0)).astype(np.float32)
        return (z * np.float32(c1)).astype(np.float32) - np.float32(c2)

    def _refB(in0, in1, c0, c1, c2):
        t = in1.astype(np.float32)
        z = (t + np.float32(c0)).astype(np.float32)
        nf = (z - np.float32(c0)).astype(np.float32)
        f = (t - nf).astype(np.float32)
        p = (((f + np.float32(c1)) * f).astype(np.float32) + np.float32(c2)).astype(
            np.float32
        )
        return (p * in0.astype(np.float32)).astype(np.float32)

    specA = Spec(body=((Src0 + C0) * C1) - C2, reference=_refA)
    _z = Src1 + C0
    _f = Src1 - (_z - C0)
    specB = Spec(body=(((_f + C1) * _f) + C2) * Src0, reference=_refB)

    def _reg(name, spec):
        if name in dve_ops._SUB_OPCODE_FOR_NAME:
            return next(op for op in dve_ops.OPS if op.name == name)
        row = dve_ops._CUSTOM_DVE_ROW_BASE + len(dve_ops.OPS)
        assert row < 0x20
        dve_ops._SUB_OPCODE_FOR_NAME[name] = row
        rd1 = _has_src1(spec)
        shas = {}
        for ver in ("v3", "v4"):
            try:
                s = DveOpSpec(
                    name=name, opcode=row, uops=lower(spec, ver=ver), rd1_en=rd1
                )
                shas[ver] = s.sha(ver)
            except Exception:
                pass
        op = DveOp(name, spec, subdim=False, uops_sha=shas)
        dve_ops.OPS.append(op)
        dve_ops.CUSTOM_DVE_SPECS[name] = spec
        return op

    _ops_registered["A"] = _reg("EXP2N_BITS_ANT", specA)
    _ops_registered["B"] = _reg("EXP2F_MUL_ANT", specB)
    from concourse.dve_ops import RECIPROCAL_APPROX_FAST

    _ops_registered["RECIP"] = RECIPROCAL_APPROX_FAST
    return _ops_registered


def build_nc(
    L=L,
    C=C,
    D=D,
    reps=1,
    ablate=(),
    st_bufs=None,
    e_bufs=5,
    g_bufs=2,
    pv_bufs=1,
    op_bufs=None,
    op_at=None,
    dve_pairs=(),
    col_split=0,
    split_pv=False,
    wide=False,
    yt_eng="dve",
    ao_eng="dve",
    v2=False,
    dve_set=(2, 6, 10, 14),
    op_slots=(6, 10, 14),
    lookahead=2,
    s1_split=True,
    s2_batch=True,
    v3=True,
    v3_pattern=("a",) * 10,
    v3_op_groups=(3, 5, 7, 9),
    cross_mode="ls",
    s2_copy_split=False,
    rec_mode="pe",
    unroll=2,
    s1_wide=False,
):
    # PSUM budget (8 banks of 2KB): st tiles are 2 banks each; pv is 2 banks
    # wide / 1 bank narrow; op (out-proj) tiles 1 bank each.
    if st_bufs is None:
        st_bufs = 2 if (wide or v3) else 3
    if op_bufs is None:
        op_bufs = 2 if wide else 1
    if op_at is None:
        op_at = 20 if wide else 10
    import contextlib
    import concourse.bacc as bacc
    import concourse.mybir as mybir
    import concourse.tile as tile

    ops = _register_dve_ops()
    OPA, OPB, OPR = ops["A"], ops["B"], ops["RECIP"]

    f32 = mybir.dt.float32
    bf16 = mybir.dt.bfloat16
    i16 = mybir.dt.int16
    Exp = mybir.ActivationFunctionType.Exp
    Copy = mybir.ActivationFunctionType.Copy

    CT = C // P          # contraction tiles over channels (4)
    LT = L // P          # key tiles (32)
    NSL = L // 512       # 512-wide l-slices (8)
    NPAIR = LT // 2      # key tile pairs per slice (16)

    nc = bacc.Bacc("TRN2", target_bir_lowering=False, debug=False)

    xt_d = nc.dram_tensor("xt", [C, L], bf16, kind="ExternalInput")
    wqk_d = nc.dram_tensor("wqk", [C, P], bf16, kind="ExternalInput")
    wv_d = nc.dram_tensor("wv", [C, D], bf16, kind="ExternalInput")
    wo_d = nc.dram_tensor("wo", [D, C], bf16, kind="ExternalInput")
    y_d = nc.dram_tensor("y", [L, C], f32, kind="ExternalOutput")

    with tile.TileContext(nc) as tc:
        with (
            tc.tile_pool(name="const", bufs=1) as constp,
            tc.tile_pool(name="xtp", bufs=1) as xtp,
            tc.tile_pool(name="qkv", bufs=1) as qkvp,
            tc.tile_pool(name="exps", bufs=e_bufs) as expp,
            tc.tile_pool(name="e2", bufs=3) as e2p,
            tc.tile_pool(name="gp", bufs=g_bufs) as gp,
            tc.tile_pool(name="aon", bufs=4) as aop,
            tc.tile_pool(name="rowp", bufs=4) as rowp,
            tc.tile_pool(name="yp", bufs=4) as yp,
            tc.tile_pool(name="drs", bufs=2, space="DRAM") as drsp,
            tc.tile_pool(name="st_ps", bufs=st_bufs, space="PSUM") as stps,
            tc.tile_pool(name="pv_ps", bufs=pv_bufs, space="PSUM") as pvps,
            tc.tile_pool(name="op_ps", bufs=op_bufs, space="PSUM") as opps,
        ):
            # ---- load inputs to SBUF
            xt_sb = []
            for ct in range(CT):
                t = xtp.tile([P, L], bf16, name=f"xt{ct}", tag=f"xt{ct}")
                nc.sync.dma_start(t[:], xt_d[ct * P : (ct + 1) * P, :])
                xt_sb.append(t)
            wqk_sb = constp.tile([P, CT, P], bf16, name="wqk_sb", tag="wqk")
            wv_sb = constp.tile([P, CT, D], bf16, name="wv_sb", tag="wv")
            for ct in range(CT):
                nc.sync.dma_start(wqk_sb[:, ct, :], wqk_d[ct * P : (ct + 1) * P, :])
                nc.sync.dma_start(wv_sb[:, ct, :], wv_d[ct * P : (ct + 1) * P, :])
            wo_sb = constp.tile([D, C], bf16, name="wo_sb", tag="wo")
            nc.sync.dma_start(wo_sb[:], wo_d[:])
            bias_t = constp.tile([P, 1], f32, name="bias_t", tag="bias")
            nc.vector.memset(bias_t[:], LNK)
            ones1 = constp.tile([1, 1], f32, name="ones1", tag="ones1")
            nc.vector.memset(ones1[:], 1.0)

            qkT = qkvp.tile([P, L], bf16, name="qkT", tag="qkT")
            if v3 and ("exp" in ablate or "st" in ablate):
                e_shared = qkvp.tile([P, 1536], bf16, name="e_shared", tag="esh")
                nc.vector.memset(e_shared[:], 0.001)
            kqT = qkvp.tile([P, L], bf16, name="kqT", tag="kqT")
            v_sb = qkvp.tile([P, LT, D + 1], bf16, name="v_sb", tag="v")
            rec_all = qkvp.tile([P, LT], f32, name="rec_all", tag="rec_all")

            u = unroll
            while u > 1 and (reps < u or reps % u != 0):
                u //= 2
            nrep = reps // u
            rep_ctx = tc.For_i(0, nrep, 1) if nrep > 1 else contextlib.nullcontext()

            def _rep_body():
              # ---- stage 1: qkT = [q;k], crossed copy kqT = [k;q]  [128, L]
              s1w = 1024 if (wide or (v3 and s1_wide)) else 512
              for ls in range(L // s1w):
                sl = slice(ls * s1w, (ls + 1) * s1w)
                ps1 = stps.tile([P, 1024], f32, name="ps1", tag="st")
                for half in range(s1w // 512):
                    hsl = slice(ls * s1w + half * 512, ls * s1w + (half + 1) * 512)
                    for ct in range(CT):
                        nc.tensor.matmul(
                            ps1[:, half * 512 : (half + 1) * 512],
                            wqk_sb[:, ct, :],
                            xt_sb[ct][:, hsl],
                            start=(ct == 0),
                            stop=(ct == CT - 1),
                        )
                if (v2 or v3) and s1_split and (ls % 2 == 1):
                    nc.vector.tensor_copy(qkT[:, sl], ps1[:, :s1w])
                else:
                    nc.scalar.activation(qkT[:, sl], ps1[:, :s1w], Copy)
                # crossed copy (partition swap) building kqT = [k; q]
                if cross_mode == "engine":
                    # engine copies with shifted output base partition, read
                    # straight from the stage-1 PSUM (no SBUF round trip)
                    if ls % 2 == 1:
                        nc.scalar.activation(kqT[:D, sl], ps1[D:P, :s1w], Copy)
                        nc.vector.tensor_copy(kqT[D:, sl], ps1[:D, :s1w])
                    else:
                        nc.vector.tensor_copy(kqT[:D, sl], ps1[D:P, :s1w])
                        nc.scalar.activation(kqT[D:, sl], ps1[:D, :s1w], Copy)
                elif cross_mode == "ls":
                    nc.sync.dma_start(kqT[:D, sl], qkT[D:, sl])
                    if not wide:
                        nc.sync.dma_start(kqT[D:, sl], qkT[:D, sl])
                elif cross_mode == "big":
                    pass  # emitted after the ls loop
                else:
                    raise ValueError(cross_mode)
              if cross_mode == "big":
                nc.sync.dma_start(kqT[:D, :], qkT[D:, :])
                if not wide:
                    nc.sync.dma_start(kqT[D:, :], qkT[:D, :])

              # ---- stage 2: v [L, D] bf16 (+ ones column for row-sums)
              if (v2 or v3) and s2_batch:
                # 16 key tiles' v per PSUM tile (each [128, 64] matmul output
                # stays inside one bank: 8 x 256B per 2KB bank), one batched
                # DVE eviction per half
                for half in range(2):
                    ps2 = stps.tile([P, 1024], f32, name="ps2", tag="st")
                    for lt16 in range(16):
                        lt = half * 16 + lt16
                        for ct in range(CT):
                            nc.tensor.matmul(
                                ps2[:, lt16 * D : (lt16 + 1) * D],
                                xt_sb[ct][:, lt * P : (lt + 1) * P],
                                wv_sb[:, ct, :],
                                start=(ct == 0),
                                stop=(ct == CT - 1),
                            )
                    if s2_copy_split and half == 0:
                        nc.scalar.activation(
                            v_sb[:, half * 16 : (half + 1) * 16, :D],
                            ps2[:].rearrange("p (t d) -> p t d", d=D),
                            Copy,
                        )
                    else:
                        nc.vector.tensor_copy(
                            v_sb[:, half * 16 : (half + 1) * 16, :D],
                            ps2[:].rearrange("p (t d) -> p t d", d=D),
                        )
              else:
                for lt in range(LT):
                  ps2 = stps.tile([P, 1024], f32, name="ps2", tag="st")
                  for ct in range(CT):
                    nc.tensor.matmul(
                        ps2[:, :D],
                        xt_sb[ct][:, lt * P : (lt + 1) * P],
                        wv_sb[:, ct, :],
                        start=(ct == 0),
                        stop=(ct == CT - 1),
                    )
                  nc.vector.tensor_copy(v_sb[:, lt, :D], ps2[:, :D])
              nc.vector.memset(v_sb[:, :, D], 1.0)

              # ---- attention per query slice (512-wide, or 1024-wide)
              pending_outproj = [None]
              ntl = 8 if wide else 4  # out-proj l-tiles per slice

              def emit_outproj(isl, ao0, ao1):
                  def emit():
                      for tloc in range(ntl):
                          t_ = isl * ntl + tloc
                          cs = slice(tloc * P, (tloc + 1) * P)
                          pp = opps.tile([P, 512], f32, name="pp", tag="op")
                          if split_pv:
                              nc.tensor.matmul(
                                  pp[:], ao0[:, cs], wo_sb[:], start=True, stop=False
                              )
                              nc.tensor.matmul(
                                  pp[:], ao1[:, cs], wo_sb[:], start=False, stop=True
                              )
                          else:
                              nc.tensor.matmul(
                                  pp[:], ao0[:, cs], wo_sb[:], start=True, stop=True
                              )
                          yt = yp.tile([P, C], f32, name="yt", tag="y")
                          use_dve = yt_eng == "dve" or (
                              yt_eng == "mix" and tloc % 2 == 0
                          )
                          if use_dve:
                              nc.vector.tensor_scalar_mul(
                                  yt[:], pp[:], rec_all[:, t_ : t_ + 1]
                              )
                          else:
                              nc.scalar.activation(
                                  yt[:], pp[:], Copy, scale=rec_all[:, t_ : t_ + 1]
                              )
                          if "ydma" not in ablate:
                              nc.sync.dma_start(y_d[t_ * P : (t_ + 1) * P, :], yt[:])

                  return emit

              if wide:
                for ws in range(L // 1024):
                    wsx = slice(ws * 1024, (ws + 1) * 1024)
                    pvw = pvps.tile([D + 1, 1024], f32, name="pvw", tag="pv")
                    for j in range(LT):
                        stp = stps.tile([P, 1024], f32, name="stp", tag="st")
                        if "st" not in ablate:
                            nc.tensor.matmul(
                                stp[:],
                                kqT[:D, j * P : (j + 1) * P],
                                qkT[:D, wsx],
                                start=True,
                                stop=True,
                            )
                        e = expp.tile([P, 1024], bf16, name="e", tag="e")
                        if "exp" not in ablate:
                            if col_split:
                                c = col_split
                                nc.scalar.activation(
                                    e[:, :c], stp[:, :c], Exp,
                                    bias=bias_t[:], scale=LN2,
                                )
                                g = gp.tile([P, 1024 - c], i16, name="g", tag="g")
                                nc.vector._custom_dve(
                                    OPA, out=g[:], in0=stp[:, c:],
                                    s0=MAGIC, s1=A_SCALE, imm2=A_BIAS,
                                )
                                nc.vector._custom_dve(
                                    OPB, out=e[:, c:], in0=g[:].bitcast(bf16),
                                    in1=stp[:, c:],
                                    s0=MAGIC, s1=POLY_B, imm2=POLY_C,
                                )
                            else:
                                nc.scalar.activation(
                                    e[:], stp[:], Exp, bias=bias_t[:], scale=LN2
                                )
                        if "pv" not in ablate:
                            nc.tensor.matmul(
                                pvw[:], v_sb[:, j, :], e[:],
                                start=(j == 0), stop=(j == LT - 1),
                            )
                        if j == op_at and pending_outproj[0] is not None:
                            pending_outproj[0]()
                            pending_outproj[0] = None
                    if "tail" in ablate:
                        continue
                    rsum = rowp.tile([1, 1024], f32, name="rsum", tag="rr")
                    nc.vector.tensor_copy(rsum[:], pvw[D : D + 1, :])
                    rec_row = rowp.tile([1, 1024], f32, name="rec_row", tag="rr")
                    nc.vector._custom_dve(
                        OPR, out=rec_row[:], in0=rsum[:],
                        s0=RECIP_S0, s1=RECIP_S1, imm2=RECIP_IMM2,
                    )
                    dr = drsp.tile([1024], f32, name="dr", tag="dr")
                    nc.sync.dma_start(dr[:], rec_row[:])
                    nc.sync.dma_start(
                        rec_all[:, ws * 8 : (ws + 1) * 8],
                        dr.rearrange("(t p) -> p t", p=P),
                    )
                    ao0 = aop.tile([D, 1024], bf16, name="ao0", tag="ao")
                    nc.vector.tensor_copy(ao0[:], pvw[:D, :])
                    if "proj" not in ablate:
                        pending_outproj[0] = emit_outproj(ws, ao0, None)
                if pending_outproj[0] is not None:
                    pending_outproj[0]()
                    pending_outproj[0] = None

              # ---- v2 attention: lookahead emission so the in-order PE
              # queue never head-of-line blocks on an in-flight exp, 4 DVE
              # exp pairs per slice to unload ScalarE below the PE pace, and
              # the out-projection spread one tile per op_slot.
              # ---- v3 attention: 3-key-tile score groups in [128,1536]
              # 3-bank PSUM tiles (st_bufs=2).  "a" groups: one ScalarE exp
              # over all 1536 cols (amortized access-latency init); "s"
              # groups: ScalarE takes tiles 0-1 (1024 cols), DVE takes tile 2
              # as a short 512-col 2-op chunk whose ~1.4us latency fits the
              # 2-group pv lookahead window.  pv emitted per single key tile.
              if v3 and not wide:
               pending_v3 = [None]
               nop3 = [0]

               def emit_op_tile_v3():
                   if pending_v3[0] is None:
                       return
                   isl_, ao_ = pending_v3[0]
                   t = nop3[0]
                   t_ = isl_ * 4 + t
                   cs = slice(t * P, (t + 1) * P)
                   pp = opps.tile([P, 512], f32, name="pp", tag="op")
                   nc.tensor.matmul(pp[:], ao_[:, cs], wo_sb[:], start=True, stop=True)
                   yt = yp.tile([P, C], f32, name="yt", tag="y")
                   nc.vector.tensor_scalar_mul(yt[:], pp[:], rec_all[:, t_ : t_ + 1])
                   nc.sync.dma_start(y_d[t_ * P : (t_ + 1) * P, :], yt[:])
                   nop3[0] += 1
                   if nop3[0] == 4:
                       pending_v3[0] = None
                       nop3[0] = 0

               groups = []
               _j = 0
               for kind in v3_pattern:
                   groups.append((kind, [_j, _j + 1, _j + 2]))
                   _j += 3
               groups.append(("a", [_j, _j + 1]))  # trailing pair (tiles 30,31)
               assert _j + 2 == LT

               for isl in range(NSL):
                isx = slice(isl * 512, (isl + 1) * 512)
                acc0 = None
                if "pv" not in ablate:
                    acc0 = pvps.tile([D + 1, 512], f32, name="acc0", tag="pv")
                pvq = []  # FIFO of per-group pv emitters

                def emit_group(kind, tiles):
                    w = len(tiles)
                    if "exp" in ablate or "st" in ablate:
                        if "st" not in ablate:
                            stp = stps.tile([P, 1536], f32, name="stp", tag="st")
                            for i, jt in enumerate(tiles):
                                cs = slice(i * 512, (i + 1) * 512)
                                if jt % 2 == 0:
                                    nc.tensor.matmul(
                                        stp[:, cs],
                                        kqT[:D, jt * P : (jt + 1) * P],
                                        qkT[:D, isx], start=True, stop=True)
                                else:
                                    nc.tensor.matmul(
                                        stp[:, cs],
                                        qkT[D:, jt * P : (jt + 1) * P],
                                        kqT[D:, isx], start=True, stop=True)
                        refs = [
                            (jt, e_shared[:, i * 512 : (i + 1) * 512])
                            for i, jt in enumerate(tiles)
                        ]

                        def emit_pvs():
                            if "pv" in ablate:
                                return
                            for jt, e_ap in refs:
                                nc.tensor.matmul(
                                    acc0[:], v_sb[:, jt, :], e_ap,
                                    start=(jt == 0), stop=(jt == LT - 1))

                        pvq.append(emit_pvs)
                        return
                    stp = stps.tile([P, 1536], f32, name="stp", tag="st")
                    for i, jt in enumerate(tiles):
                        cs = slice(i * 512, (i + 1) * 512)
                        if jt % 2 == 0:
                            nc.tensor.matmul(
                                stp[:, cs],
                                kqT[:D, jt * P : (jt + 1) * P],
                                qkT[:D, isx],
                                start=True, stop=True,
                            )
                        else:
                            nc.tensor.matmul(
                                stp[:, cs],
                                qkT[D:, jt * P : (jt + 1) * P],
                                kqT[D:, isx],
                                start=True, stop=True,
                            )
                    refs = []  # (key tile, e AP) for the pv stage
                    if kind == "s" and w == 3:
                        ea = expp.tile([P, 1536], bf16, name="e", tag="e")
                        nc.scalar.activation(
                            ea[:, :1024], stp[:, :1024], Exp,
                            bias=bias_t[:], scale=LN2,
                        )
                        ed = e2p.tile([P, 512], bf16, name="ed", tag="ed")
                        g = gp.tile([P, 512], i16, name="g", tag="g")
                        nc.vector._custom_dve(
                            OPA, out=g[:], in0=stp[:, 1024:],
                            s0=MAGIC, s1=A_SCALE, imm2=A_BIAS,
                        )
                        nc.vector._custom_dve(
                            OPB, out=ed[:], in0=g[:].bitcast(bf16),
                            in1=stp[:, 1024:],
                            s0=MAGIC, s1=POLY_B, imm2=POLY_C,
                        )
                        refs = [
                            (tiles[0], ea[:, :512]),
                            (tiles[1], ea[:, 512:1024]),
                            (tiles[2], ed[:]),
                        ]
                    else:
                        ea = expp.tile([P, 1536], bf16, name="e", tag="e")
                        nc.scalar.activation(
                            ea[:, : w * 512], stp[:, : w * 512], Exp,
                            bias=bias_t[:], scale=LN2,
                        )
                        refs = [
                            (jt, ea[:, i * 512 : (i + 1) * 512])
                            for i, jt in enumerate(tiles)
                        ]

                    def emit_pvs():
                        for jt, e_ap in refs:
                            nc.tensor.matmul(
                                acc0[:], v_sb[:, jt, :], e_ap,
                                start=(jt == 0), stop=(jt == LT - 1),
                            )

                    pvq.append(emit_pvs)

                for gi, (kind, tiles) in enumerate(groups):
                    emit_group(kind, tiles)
                    if gi >= 2:
                        pvq.pop(0)()
                    if gi in v3_op_groups:
                        emit_op_tile_v3()
                while pvq:
                    pvq.pop(0)()
                emit_op_tile_v3()  # 4th out-proj tile of the previous slice

                if "tail" in ablate:
                    continue
                rsum = rowp.tile([1, 512], f32, name="rsum", tag="rr")
                nc.vector.tensor_copy(rsum[:], acc0[D : D + 1, :])
                if rec_mode == "pe":
                    # transpose the row sums [1,512] -> [128,4] with 4 tiny
                    # PE transposes, reciprocal straight into rec_all: no
                    # DRAM bounce on the yt critical path
                    tp = opps.tile([P, 512], f32, name="tp", tag="op")
                    for t in range(4):
                        nc.tensor.matmul(
                            tp[:, t : t + 1],
                            rsum[:, t * P : (t + 1) * P],
                            ones1[:],
                            is_transpose=True, start=True, stop=True,
                        )
                    nc.vector._custom_dve(
                        OPR, out=rec_all[:, isl * 4 : (isl + 1) * 4],
                        in0=tp[:, :4],
                        s0=RECIP_S0, s1=RECIP_S1, imm2=RECIP_IMM2,
                    )
                else:
                    rec_row = rowp.tile([1, 512], f32, name="rec_row", tag="rr")
                    nc.vector._custom_dve(
                        OPR, out=rec_row[:], in0=rsum[:],
                        s0=RECIP_S0, s1=RECIP_S1, imm2=RECIP_IMM2,
                    )
                    dr = drsp.tile([512], f32, name="dr", tag="dr")
                    nc.sync.dma_start(dr[:], rec_row[:])
                    nc.sync.dma_start(
                        rec_all[:, isl * 4 : (isl + 1) * 4],
                        dr.rearrange("(t p) -> p t", p=P),
                    )
                ao0 = aop.tile([D, 512], bf16, name="ao0", tag="ao")
                if ao_eng == "scalar":
                    nc.scalar.activation(ao0[:], acc0[:D, :], Copy)
                else:
                    nc.vector.tensor_copy(ao0[:], acc0[:D, :])
                if "proj" not in ablate:
                    pending_v3[0] = (isl, ao0)

               while pending_v3[0] is not None:
                   emit_op_tile_v3()

              if v2 and not wide:
               pending_v2 = [None]  # (isl, ao0) of the slice awaiting outproj
               nop = [0]

               def emit_op_tile_v2():
                   if pending_v2[0] is None:
                       return
                   isl_, ao_ = pending_v2[0]
                   t = nop[0]
                   t_ = isl_ * 4 + t
                   cs = slice(t * P, (t + 1) * P)
                   pp = opps.tile([P, 512], f32, name="pp", tag="op")
                   nc.tensor.matmul(pp[:], ao_[:, cs], wo_sb[:], start=True, stop=True)
                   yt = yp.tile([P, C], f32, name="yt", tag="y")
                   nc.vector.tensor_scalar_mul(yt[:], pp[:], rec_all[:, t_ : t_ + 1])
                   nc.sync.dma_start(y_d[t_ * P : (t_ + 1) * P, :], yt[:])
                   nop[0] += 1
                   if nop[0] == 4:
                       pending_v2[0] = None
                       nop[0] = 0

               for isl in range(NSL):
                isx = slice(isl * 512, (isl + 1) * 512)
                acc0 = pvps.tile([D + 1, 512], f32, name="acc0", tag="pv")
                stq = {}
                eq = {}

                def emit_scores_v2(m):
                    jA, jB = 2 * m, 2 * m + 1
                    stp = stps.tile([P, 1024], f32, name="stp", tag="st")
                    nc.tensor.matmul(
                        stp[:, :512],
                        kqT[:D, jA * P : (jA + 1) * P],
                        qkT[:D, isx],
                        start=True,
                        stop=True,
                    )
                    nc.tensor.matmul(
                        stp[:, 512:],
                        qkT[D:, jB * P : (jB + 1) * P],
                        kqT[D:, isx],
                        start=True,
                        stop=True,
                    )
                    stq[m] = stp

                def emit_exp_v2(m):
                    stp = stq[m]
                    e = expp.tile([P, 1024], bf16, name="e", tag="e")
                    if m in dve_set:
                        g = gp.tile([P, 1024], i16, name="g", tag="g")
                        nc.vector._custom_dve(
                            OPA, out=g[:], in0=stp[:],
                            s0=MAGIC, s1=A_SCALE, imm2=A_BIAS,
                        )
                        nc.vector._custom_dve(
                            OPB, out=e[:], in0=g[:].bitcast(bf16), in1=stp[:],
                            s0=MAGIC, s1=POLY_B, imm2=POLY_C,
                        )
                    else:
                        nc.scalar.activation(
                            e[:], stp[:], Exp, bias=bias_t[:], scale=LN2
                        )
                    eq[m] = e

                def emit_pv_v2(p):
                    jA, jB = 2 * p, 2 * p + 1
                    e = eq.pop(p)
                    stq.pop(p)
                    nc.tensor.matmul(
                        acc0[:], v_sb[:, jA, :], e[:, :512],
                        start=(p == 0), stop=False,
                    )
                    nc.tensor.matmul(
                        acc0[:], v_sb[:, jB, :], e[:, 512:],
                        start=False, stop=(p == NPAIR - 1),
                    )

                for m in range(NPAIR):
                    emit_scores_v2(m)
                    emit_exp_v2(m)
                    if m >= lookahead:
                        emit_pv_v2(m - lookahead)
                    if m in op_slots:
                        emit_op_tile_v2()
                for p in range(NPAIR - lookahead, NPAIR):
                    emit_pv_v2(p)
                emit_op_tile_v2()  # 4th out-proj tile of the previous slice

                # slice tail: denominators + reciprocal + ao eviction
                rsum = rowp.tile([1, 512], f32, name="rsum", tag="rr")
                nc.vector.tensor_copy(rsum[:], acc0[D : D + 1, :])
                rec_row = rowp.tile([1, 512], f32, name="rec_row", tag="rr")
                nc.vector._custom_dve(
                    OPR, out=rec_row[:], in0=rsum[:],
                    s0=RECIP_S0, s1=RECIP_S1, imm2=RECIP_IMM2,
                )
                dr = drsp.tile([512], f32, name="dr", tag="dr")
                nc.sync.dma_start(dr[:], rec_row[:])
                nc.sync.dma_start(
                    rec_all[:, isl * 4 : (isl + 1) * 4],
                    dr.rearrange("(t p) -> p t", p=P),
                )
                ao0 = aop.tile([D, 512], bf16, name="ao0", tag="ao")
                if ao_eng == "scalar":
                    nc.scalar.activation(ao0[:], acc0[:D, :], Copy)
                else:
                    nc.vector.tensor_copy(ao0[:], acc0[:D, :])
                pending_v2[0] = (isl, ao0)

               while pending_v2[0] is not None:
                   emit_op_tile_v2()

              if not wide and not v2 and not v3:
               for isl in range(NSL):
                isx = slice(isl * 512, (isl + 1) * 512)
                if split_pv:
                    acc0 = pvps.tile([D + 1, 512], f32, name="acc0", tag="pv")
                    acc1 = pvps.tile([D + 1, 512], f32, name="acc1", tag="pv")
                else:
                    acc0 = pvps.tile([D + 1, 512], f32, name="acc0", tag="pv")
                    acc1 = None
                for m in range(NPAIR):
                    jA, jB = 2 * m, 2 * m + 1
                    stp = stps.tile([P, 1024], f32, name="stp", tag="st")
                    if "st" not in ablate:
                        nc.tensor.matmul(
                            stp[:, :512],
                            kqT[:D, jA * P : (jA + 1) * P],
                            qkT[:D, isx],
                            start=True,
                            stop=True,
                        )
                        nc.tensor.matmul(
                            stp[:, 512:],
                            qkT[D:, jB * P : (jB + 1) * P],
                            kqT[D:, isx],
                            start=True,
                            stop=True,
                        )
                    e = expp.tile([P, 1024], bf16, name="e", tag="e")
                    if "exp" not in ablate:
                        if col_split:
                            c = col_split
                            nc.scalar.activation(
                                e[:, :c], stp[:, :c], Exp, bias=bias_t[:], scale=LN2
                            )
                            g = gp.tile([P, 1024 - c], i16, name="g", tag="g")
                            nc.vector._custom_dve(
                                OPA, out=g[:], in0=stp[:, c:],
                                s0=MAGIC, s1=A_SCALE, imm2=A_BIAS,
                            )
                            nc.vector._custom_dve(
                                OPB, out=e[:, c:], in0=g[:].bitcast(bf16),
                                in1=stp[:, c:],
                                s0=MAGIC, s1=POLY_B, imm2=POLY_C,
                            )
                        elif m in dve_pairs:
                            g = gp.tile([P, 1024], i16, name="g", tag="g")
                            nc.vector._custom_dve(
                                OPA, out=g[:], in0=stp[:],
                                s0=MAGIC, s1=A_SCALE, imm2=A_BIAS,
                            )
                            nc.vector._custom_dve(
                                OPB, out=e[:], in0=g[:].bitcast(bf16), in1=stp[:],
                                s0=MAGIC, s1=POLY_B, imm2=POLY_C,
                            )
                        else:
                            nc.scalar.activation(
                                e[:], stp[:], Exp, bias=bias_t[:], scale=LN2
                            )
                    if "pv" not in ablate:
                        if split_pv:
                            nc.tensor.matmul(
                                acc0[:], v_sb[:D, jA, :], e[:D, :512],
                                start=(m == 0), stop=False,
                            )
                            nc.tensor.matmul(
                                acc1[:], v_sb[D:, jA, :], e[D:, :512],
                                start=(m == 0), stop=False,
                            )
                            nc.tensor.matmul(
                                acc0[:], v_sb[:D, jB, :], e[:D, 512:],
                                start=False, stop=(m == NPAIR - 1),
                            )
                            nc.tensor.matmul(
                                acc1[:], v_sb[D:, jB, :], e[D:, 512:],
                                start=False, stop=(m == NPAIR - 1),
                            )
                        else:
                            nc.tensor.matmul(
                                acc0[:], v_sb[:, jA, :], e[:, :512],
                                start=(m == 0), stop=False,
                            )
                            nc.tensor.matmul(
                                acc0[:], v_sb[:, jB, :], e[:, 512:],
                                start=False, stop=(m == NPAIR - 1),
                            )
                    if m == op_at and pending_outproj[0] is not None:
                        pending_outproj[0]()
                        pending_outproj[0] = None

                if "tail" in ablate:
                    continue
                # slice tail: denominators + reciprocal + ao eviction
                rsum = rowp.tile([1, 512], f32, name="rsum", tag="rr")
                nc.vector.tensor_copy(rsum[:], acc0[D : D + 1, :])
                if split_pv:
                    # DVE reads at most one PSUM input: rsum is SBUF now
                    nc.vector.tensor_add(rsum[:], rsum[:], acc1[D : D + 1, :])
                rec_row = rowp.tile([1, 512], f32, name="rec_row", tag="rr")
                nc.vector._custom_dve(
                    OPR, out=rec_row[:], in0=rsum[:],
                    s0=RECIP_S0, s1=RECIP_S1, imm2=RECIP_IMM2,
                )
                dr = drsp.tile([512], f32, name="dr", tag="dr")
                nc.sync.dma_start(dr[:], rec_row[:])
                nc.sync.dma_start(
                    rec_all[:, isl * 4 : (isl + 1) * 4],
                    dr.rearrange("(t p) -> p t", p=P),
                )
                ao0 = aop.tile([D, 512], bf16, name="ao0", tag="ao")
                if ao_eng == "scalar":
                    nc.scalar.activation(ao0[:], acc0[:D, :], Copy)
                else:
                    nc.vector.tensor_copy(ao0[:], acc0[:D, :])
                if split_pv:
                    ao1 = aop.tile([D, 512], bf16, name="ao1", tag="ao")
                    nc.vector.tensor_copy(ao1[:], acc1[:D, :])
                else:
                    ao1 = None
                if "proj" not in ablate:
                    pending_outproj[0] = emit_outproj(isl, ao0, ao1)

              if pending_outproj[0] is not None:
                  pending_outproj[0]()
                  pending_outproj[0] = None

            with rep_ctx:
                for _uu in range(u):
                    _rep_body()

    nc.compile()
    return nc


_nc_cache = {}


def _get_nc(**kw):
    key = tuple(sorted(kw.items()))
    if key not in _nc_cache:
        _nc_cache[key] = build_nc(**kw)
    return _nc_cache[key]


def make_in_maps(x, w_qkv, w_out):
    """Host-side sharding: per-head weight slices, shared transposed input.
    wq is pre-scaled by D^-1/2 * log2(e): scores arrive as t = s*log2e, so
    e^s == 2^t (ScalarE exp uses scale=ln2 to undo; the DVE path computes
    2^t directly)."""
    x = np.asarray(x, dtype=np.float32)
    w_qkv = np.asarray(w_qkv, dtype=np.float32)
    w_out = np.asarray(w_out, dtype=np.float32)
    scale = float(D) ** -0.5 * LOG2E
    xt = np.ascontiguousarray(x[0].T).astype(_BF16)  # [C, L]
    in_maps = []
    for h in range(N_CORES):
        sl = slice(h * D, (h + 1) * D)
        wq = (w_qkv[0 * C :][sl, :] * scale).T  # [C, D]
        wk = w_qkv[1 * C :][sl, :].T
        wqk = np.ascontiguousarray(np.concatenate([wq, wk], axis=1)).astype(_BF16)
        wv = np.ascontiguousarray(w_qkv[2 * C :][sl, :].T).astype(_BF16)
        wo = np.ascontiguousarray(w_out[:, sl].T).astype(_BF16)
        in_maps.append({"xt": xt, "wqk": wqk, "wv": wv, "wo": wo})
    return in_maps


def kernel(x, w_qkv, w_out, b_out):
    from concourse.bass_utils import run_bass_kernel_spmd

    nc = _get_nc()
    in_maps = make_in_maps(x, w_qkv, w_out)
    res = run_bass_kernel_spmd(nc, in_maps, list(range(N_CORES)))
    y = res.results[0]["y"].copy()
    for i in range(1, N_CORES):
        y += res.results[i]["y"]
    y += np.asarray(b_out, dtype=np.float32)
    return y[None]



# revision 22
# speedup vs baseline: 1.1863x; 1.1863x over previous
"""Multi-head attention (B=1, L=4096, C=512, H=8, D=64) on 8 TRN2 NeuronCores.

Sharding: head-parallel - core h computes head h end-to-end (QKV projection
for its head, softmax attention, its partial out-projection). Host sums the
8 partial projections and adds the bias.

Default path is "v3" (this session's winner; v1/v2 kept behind flags):
  * stage 1: psum [q;k] = [wq|wk].T @ xT-slices, PSUM->SBUF copies
    alternating ScalarE/DVE; crossed SBUF->SBUF DMA builds kqT so paired
    score matmuls have aligned base partitions.  wq pre-scaled by
    D^-1/2*log2(e) on the host (scores arrive as t = s*log2e).
  * stage 2: v[L,D] bf16 + ones column (PV then accumulates softmax
    denominators for free in accumulator row D); 16 key tiles' v matmuls
    batched per PSUM tile, ONE strided DVE eviction per half.
  * attention per 512-wide query slice: 3-key-tile score GROUPS in
    [128,1536] 3-bank PSUM tiles (st_bufs=2), ONE ScalarE exp instr per
    group (amortizes the ~222cy access-latency init over 1536 cols), pv
    emitted per single key tile with a 2-group lookahead (PE stream:
    scores(g), pv-triplet(g-2)) so the in-order PE queue never head-of-line
    blocks on an in-flight exp.  Out-proj of slice i-1 spread ONE tile per
    op group {3,5,7,9} of slice i (op_bufs=1: spreading avoids the
    pp->yt->pp serialization stall of emitting 4 tiles back-to-back).
  * slice tail: row sums transposed [1,512]->[128,4] by 4 tiny PE
    transpose matmuls into an op-pool PSUM tile, then the 1-pass DVE
    reciprocal seed op writes rec_all directly (rec_mode="pe"; the old
    DRAM-bounce transpose put 2 DMA round trips on the yt critical path).
  * PSUM budget (8 banks): 2 score groups (3 banks each) + 1 PV + 1 op.
  * Measured-on-HW notes (axon wall-deltas are +/-15%; within-batch
    comparisons only): the custom 2-op DVE exp (OPA/OPB below) is
    throughput-priced as modeled (~1.37us per 512-col chunk) but ANY
    mixing of it into the exp stream measured slower end-to-end (+7us at
    2 chunks/slice, +21us all-DVE) - the PV dependency chain pays its
    latency and the kernel sits at a PE/ScalarE equilibrium (~16us/slice
    effective, consistent with partially-exposed LoadStationary, "2.1GHz
    effective PE").  fp8 DoubleRow (2x PE) is numerically DEAD here:
    e4m3 on v alone costs 1.7e-2 absmax rel err (budget 2e-2), scores
    5.8e-2.  ScalarE Exp and Copy share an act table (no reload thrash).
  * unroll=2 (two passes per For_i iteration) measured -11us/pass within
    batch: the hardware loop boundary costs ~20us/iteration.  unroll=3/4/8
    all measured WORSE than unroll=2 (larger loop bodies appear to
    thrash sequencer instruction fetch) - 2 is the sweet spot.
  * the T0/T8 quadrant ALTERNATION of the paired score forms is
    load-bearing: score_form="a" (all matmuls in one quadrant, half the
    crossed DMAs) measured +26us - alternating row-quadrants lets the
    next matmul's LoadStationary overlap the current one's execution.
"""

import numpy as np
import ml_dtypes

L, C, D, H = 4096, 512, 64, 8
N_CORES = 8
P = 128

_BF16 = ml_dtypes.bfloat16

# ---- custom DVE exp: constants -------------------------------------------
MAGIC = 12582527.0          # 2^23 + 512k + 127: magic round-to-int addend
POLY_B = 2.9504             # p(f) = (f + B)*f + C  ~  K * 2^f  on [-.5, .5]
POLY_C = 4.19605
POLY_K = 4.194189908867873
A_SCALE = 128.0
A_BIAS = (MAGIC - 127.0) * 128.0
LOG2E = 1.4426950408889634
LN2 = 0.6931471805599453
LNK = float(np.log(POLY_K))

# reciprocal seed constants (from concourse.dve_ops.RECIP_APPROX_FAST_CONSTS)
RECIP_S0, RECIP_S1, RECIP_IMM2 = -0.23549792, 2.0017324, 2.0

_ops_registered = {}


def _register_dve_ops():
    """Register the two custom DVE exp micro-ops (runtime registration: the
    uop table is generated per-NEFF from dve_ops.OPS at compile time)."""
    if _ops_registered:
        return _ops_registered
    from concourse.dve_spec import Spec, Src0, Src1, C0, C1, C2, lower, _has_src1
    from concourse.dve_uop import DveOpSpec
    import concourse.dve_ops as dve_ops
    from concourse.dve_ops import DveOp

    def _refA(in0, in1, c0, c1, c2):
        z = (in0.astype(np.float32) + np.float32(c0)).astype(np.float32)
        return (z * np.float32(c1)).astype(np.float32) - np.float32(c2)

    def _refB(in0, in1, c0, c1, c2):
        t = in1.astype(np.float32)
        z = (t + np.float32(c0)).astype(np.float32)
        nf = (z - np.float32(c0)).astype(np.float32)
        f = (t - nf).astype(np.float32)
        p = (((f + np.float32(c1)) * f).astype(np.float32) + np.float32(c2)).astype(
            np.float32
        )
        return (p * in0.astype(np.float32)).astype(np.float32)

    specA = Spec(body=((Src0 + C0) * C1) - C2, reference=_refA)
    _z = Src1 + C0
    _f = Src1 - (_z - C0)
    specB = Spec(body=(((_f + C1) * _f) + C2) * Src0, reference=_refB)

    def _reg(name, spec):
        if name in dve_ops._SUB_OPCODE_FOR_NAME:
            return next(op for op in dve_ops.OPS if op.name == name)
        row = dve_ops._CUSTOM_DVE_ROW_BASE + len(dve_ops.OPS)
        assert row < 0x20
        dve_ops._SUB_OPCODE_FOR_NAME[name] = row
        rd1 = _has_src1(spec)
        shas = {}
        for ver in ("v3", "v4"):
            try:
                s = DveOpSpec(
                    name=name, opcode=row, uops=lower(spec, ver=ver), rd1_en=rd1
                )
                shas[ver] = s.sha(ver)
            except Exception:
                pass
        op = DveOp(name, spec, subdim=False, uops_sha=shas)
        dve_ops.OPS.append(op)
        dve_ops.CUSTOM_DVE_SPECS[name] = spec
        return op

    _ops_registered["A"] = _reg("EXP2N_BITS_ANT", specA)
    _ops_registered["B"] = _reg("EXP2F_MUL_ANT", specB)
    from concourse.dve_ops import RECIPROCAL_APPROX_FAST

    _ops_registered["RECIP"] = RECIPROCAL_APPROX_FAST
    return _ops_registered


def build_nc(
    L=L,
    C=C,
    D=D,
    reps=1,
    ablate=(),
    st_bufs=None,
    e_bufs=5,
    g_bufs=2,
    pv_bufs=1,
    op_bufs=None,
    op_at=None,
    dve_pairs=(),
    col_split=0,
    split_pv=False,
    wide=False,
    yt_eng="dve",
    ao_eng="dve",
    v2=False,
    dve_set=(2, 6, 10, 14),
    op_slots=(6, 10, 14),
    lookahead=2,
    s1_split=True,
    s2_batch=True,
    v3=True,
    v3_pattern=("a",) * 10,
    v3_op_groups=(3, 5, 7, 9),
    cross_mode="ls",
    s2_copy_split=False,
    rec_mode="pe",
    unroll=2,
    s1_wide=False,
    score_form="ab",
):
    # PSUM budget (8 banks of 2KB): st tiles are 2 banks each; pv is 2 banks
    # wide / 1 bank narrow; op (out-proj) tiles 1 bank each.
    if st_bufs is None:
        st_bufs = 2 if (wide or v3) else 3
    if op_bufs is None:
        op_bufs = 2 if wide else 1
    if op_at is None:
        op_at = 20 if wide else 10
    import contextlib
    import concourse.bacc as bacc
    import concourse.mybir as mybir
    import concourse.tile as tile

    ops = _register_dve_ops()
    OPA, OPB, OPR = ops["A"], ops["B"], ops["RECIP"]

    f32 = mybir.dt.float32
    bf16 = mybir.dt.bfloat16
    i16 = mybir.dt.int16
    Exp = mybir.ActivationFunctionType.Exp
    Copy = mybir.ActivationFunctionType.Copy

    CT = C // P          # contraction tiles over channels (4)
    LT = L // P          # key tiles (32)
    NSL = L // 512       # 512-wide l-slices (8)
    NPAIR = LT // 2      # key tile pairs per slice (16)

    nc = bacc.Bacc("TRN2", target_bir_lowering=False, debug=False)

    xt_d = nc.dram_tensor("xt", [C, L], bf16, kind="ExternalInput")
    wqk_d = nc.dram_tensor("wqk", [C, P], bf16, kind="ExternalInput")
    wv_d = nc.dram_tensor("wv", [C, D], bf16, kind="ExternalInput")
    wo_d = nc.dram_tensor("wo", [D, C], bf16, kind="ExternalInput")
    y_d = nc.dram_tensor("y", [L, C], f32, kind="ExternalOutput")

    with tile.TileContext(nc) as tc:
        with (
            tc.tile_pool(name="const", bufs=1) as constp,
            tc.tile_pool(name="xtp", bufs=1) as xtp,
            tc.tile_pool(name="qkv", bufs=1) as qkvp,
            tc.tile_pool(name="exps", bufs=e_bufs) as expp,
            tc.tile_pool(name="e2", bufs=3) as e2p,
            tc.tile_pool(name="gp", bufs=g_bufs) as gp,
            tc.tile_pool(name="aon", bufs=4) as aop,
            tc.tile_pool(name="rowp", bufs=4) as rowp,
            tc.tile_pool(name="yp", bufs=4) as yp,
            tc.tile_pool(name="drs", bufs=2, space="DRAM") as drsp,
            tc.tile_pool(name="st_ps", bufs=st_bufs, space="PSUM") as stps,
            tc.tile_pool(name="pv_ps", bufs=pv_bufs, space="PSUM") as pvps,
            tc.tile_pool(name="op_ps", bufs=op_bufs, space="PSUM") as opps,
        ):
            # ---- load inputs to SBUF
            xt_sb = []
            for ct in range(CT):
                t = xtp.tile([P, L], bf16, name=f"xt{ct}", tag=f"xt{ct}")
                nc.sync.dma_start(t[:], xt_d[ct * P : (ct + 1) * P, :])
                xt_sb.append(t)
            wqk_sb = constp.tile([P, CT, P], bf16, name="wqk_sb", tag="wqk")
            wv_sb = constp.tile([P, CT, D], bf16, name="wv_sb", tag="wv")
            for ct in range(CT):
                nc.sync.dma_start(wqk_sb[:, ct, :], wqk_d[ct * P : (ct + 1) * P, :])
                nc.sync.dma_start(wv_sb[:, ct, :], wv_d[ct * P : (ct + 1) * P, :])
            wo_sb = constp.tile([D, C], bf16, name="wo_sb", tag="wo")
            nc.sync.dma_start(wo_sb[:], wo_d[:])
            bias_t = constp.tile([P, 1], f32, name="bias_t", tag="bias")
            nc.vector.memset(bias_t[:], LNK)
            ones1 = constp.tile([1, 1], f32, name="ones1", tag="ones1")
            nc.vector.memset(ones1[:], 1.0)

            qkT = qkvp.tile([P, L], bf16, name="qkT", tag="qkT")
            if v3 and ("exp" in ablate or "st" in ablate):
                e_shared = qkvp.tile([P, 1536], bf16, name="e_shared", tag="esh")
                nc.vector.memset(e_shared[:], 0.001)
            kqT = qkvp.tile([P, L], bf16, name="kqT", tag="kqT")
            v_sb = qkvp.tile([P, LT, D + 1], bf16, name="v_sb", tag="v")
            rec_all = qkvp.tile([P, LT], f32, name="rec_all", tag="rec_all")

            u = unroll
            while u > 1 and (reps < u or reps % u != 0):
                u //= 2
            nrep = reps // u
            rep_ctx = tc.For_i(0, nrep, 1) if nrep > 1 else contextlib.nullcontext()

            def _rep_body():
              # ---- stage 1: qkT = [q;k], crossed copy kqT = [k;q]  [128, L]
              s1w = 1024 if (wide or (v3 and s1_wide)) else 512
              for ls in range(L // s1w):
                sl = slice(ls * s1w, (ls + 1) * s1w)
                ps1 = stps.tile([P, 1024], f32, name="ps1", tag="st")
                for half in range(s1w // 512):
                    hsl = slice(ls * s1w + half * 512, ls * s1w + (half + 1) * 512)
                    for ct in range(CT):
                        nc.tensor.matmul(
                            ps1[:, half * 512 : (half + 1) * 512],
                            wqk_sb[:, ct, :],
                            xt_sb[ct][:, hsl],
                            start=(ct == 0),
                            stop=(ct == CT - 1),
                        )
                if (v2 or v3) and s1_split and (ls % 2 == 1):
                    nc.vector.tensor_copy(qkT[:, sl], ps1[:, :s1w])
                else:
                    nc.scalar.activation(qkT[:, sl], ps1[:, :s1w], Copy)
                # crossed copy (partition swap) building kqT = [k; q]
                if cross_mode == "engine":
                    # engine copies with shifted output base partition, read
                    # straight from the stage-1 PSUM (no SBUF round trip)
                    if ls % 2 == 1:
                        nc.scalar.activation(kqT[:D, sl], ps1[D:P, :s1w], Copy)
                        nc.vector.tensor_copy(kqT[D:, sl], ps1[:D, :s1w])
                    else:
                        nc.vector.tensor_copy(kqT[:D, sl], ps1[D:P, :s1w])
                        nc.scalar.activation(kqT[D:, sl], ps1[:D, :s1w], Copy)
                elif cross_mode == "ls":
                    nc.sync.dma_start(kqT[:D, sl], qkT[D:, sl])
                    if not wide and not (v3 and score_form == "a"):
                        nc.sync.dma_start(kqT[D:, sl], qkT[:D, sl])
                elif cross_mode == "big":
                    pass  # emitted after the ls loop
                else:
                    raise ValueError(cross_mode)
              if cross_mode == "big":
                nc.sync.dma_start(kqT[:D, :], qkT[D:, :])
                if not wide:
                    nc.sync.dma_start(kqT[D:, :], qkT[:D, :])

              # ---- stage 2: v [L, D] bf16 (+ ones column for row-sums)
              if (v2 or v3) and s2_batch:
                # 16 key tiles' v per PSUM tile (each [128, 64] matmul output
                # stays inside one bank: 8 x 256B per 2KB bank), one batched
                # DVE eviction per half
                for half in range(2):
                    ps2 = stps.tile([P, 1024], f32, name="ps2", tag="st")
                    for lt16 in range(16):
                        lt = half * 16 + lt16
                        for ct in range(CT):
                            nc.tensor.matmul(
                                ps2[:, lt16 * D : (lt16 + 1) * D],
                                xt_sb[ct][:, lt * P : (lt + 1) * P],
                                wv_sb[:, ct, :],
                                start=(ct == 0),
                                stop=(ct == CT - 1),
                            )
                    if s2_copy_split and half == 0:
                        nc.scalar.activation(
                            v_sb[:, half * 16 : (half + 1) * 16, :D],
                            ps2[:].rearrange("p (t d) -> p t d", d=D),
                            Copy,
                        )
                    else:
                        nc.vector.tensor_copy(
                            v_sb[:, half * 16 : (half + 1) * 16, :D],
                            ps2[:].rearrange("p (t d) -> p t d", d=D),
                        )
              else:
                for lt in range(LT):
                  ps2 = stps.tile([P, 1024], f32, name="ps2", tag="st")
                  for ct in range(CT):
                    nc.tensor.matmul(
                        ps2[:, :D],
                        xt_sb[ct][:, lt * P : (lt + 1) * P],
                        wv_sb[:, ct, :],
                        start=(ct == 0),
                        stop=(ct == CT - 1),
                    )
                  nc.vector.tensor_copy(v_sb[:, lt, :D], ps2[:, :D])
              nc.vector.memset(v_sb[:, :, D], 1.0)

              # ---- attention per query slice (512-wide, or 1024-wide)
              pending_outproj = [None]
              ntl = 8 if wide else 4  # out-proj l-tiles per slice

              def emit_outproj(isl, ao0, ao1):
                  def emit():
                      for tloc in range(ntl):
                          t_ = isl * ntl + tloc
                          cs = slice(tloc * P, (tloc + 1) * P)
                          pp = opps.tile([P, 512], f32, name="pp", tag="op")
                          if split_pv:
                              nc.tensor.matmul(
                                  pp[:], ao0[:, cs], wo_sb[:], start=True, stop=False
                              )
                              nc.tensor.matmul(
                                  pp[:], ao1[:, cs], wo_sb[:], start=False, stop=True
                              )
                          else:
                              nc.tensor.matmul(
                                  pp[:], ao0[:, cs], wo_sb[:], start=True, stop=True
                              )
                          yt = yp.tile([P, C], f32, name="yt", tag="y")
                          use_dve = yt_eng == "dve" or (
                              yt_eng == "mix" and tloc % 2 == 0
                          )
                          if use_dve:
                              nc.vector.tensor_scalar_mul(
                                  yt[:], pp[:], rec_all[:, t_ : t_ + 1]
                              )
                          else:
                              nc.scalar.activation(
                                  yt[:], pp[:], Copy, scale=rec_all[:, t_ : t_ + 1]
                              )
                          if "ydma" not in ablate:
                              nc.sync.dma_start(y_d[t_ * P : (t_ + 1) * P, :], yt[:])

                  return emit

              if wide:
                for ws in range(L // 1024):
                    wsx = slice(ws * 1024, (ws + 1) * 1024)
                    pvw = pvps.tile([D + 1, 1024], f32, name="pvw", tag="pv")
                    for j in range(LT):
                        stp = stps.tile([P, 1024], f32, name="stp", tag="st")
                        if "st" not in ablate:
                            nc.tensor.matmul(
                                stp[:],
                                kqT[:D, j * P : (j + 1) * P],
                                qkT[:D, wsx],
                                start=True,
                                stop=True,
                            )
                        e = expp.tile([P, 1024], bf16, name="e", tag="e")
                        if "exp" not in ablate:
                            if col_split:
                                c = col_split
                                nc.scalar.activation(
                                    e[:, :c], stp[:, :c], Exp,
                                    bias=bias_t[:], scale=LN2,
                                )
                                g = gp.tile([P, 1024 - c], i16, name="g", tag="g")
                                nc.vector._custom_dve(
                                    OPA, out=g[:], in0=stp[:, c:],
                                    s0=MAGIC, s1=A_SCALE, imm2=A_BIAS,
                                )
                                nc.vector._custom_dve(
                                    OPB, out=e[:, c:], in0=g[:].bitcast(bf16),
                                    in1=stp[:, c:],
                                    s0=MAGIC, s1=POLY_B, imm2=POLY_C,
                                )
                            else:
                                nc.scalar.activation(
                                    e[:], stp[:], Exp, bias=bias_t[:], scale=LN2
                                )
                        if "pv" not in ablate:
                            nc.tensor.matmul(
                                pvw[:], v_sb[:, j, :], e[:],
                                start=(j == 0), stop=(j == LT - 1),
                            )
                        if j == op_at and pending_outproj[0] is not None:
                            pending_outproj[0]()
                            pending_outproj[0] = None
                    if "tail" in ablate:
                        continue
                    rsum = rowp.tile([1, 1024], f32, name="rsum", tag="rr")
                    nc.vector.tensor_copy(rsum[:], pvw[D : D + 1, :])
                    rec_row = rowp.tile([1, 1024], f32, name="rec_row", tag="rr")
                    nc.vector._custom_dve(
                        OPR, out=rec_row[:], in0=rsum[:],
                        s0=RECIP_S0, s1=RECIP_S1, imm2=RECIP_IMM2,
                    )
                    dr = drsp.tile([1024], f32, name="dr", tag="dr")
                    nc.sync.dma_start(dr[:], rec_row[:])
                    nc.sync.dma_start(
                        rec_all[:, ws * 8 : (ws + 1) * 8],
                        dr.rearrange("(t p) -> p t", p=P),
                    )
                    ao0 = aop.tile([D, 1024], bf16, name="ao0", tag="ao")
                    nc.vector.tensor_copy(ao0[:], pvw[:D, :])
                    if "proj" not in ablate:
                        pending_outproj[0] = emit_outproj(ws, ao0, None)
                if pending_outproj[0] is not None:
                    pending_outproj[0]()
                    pending_outproj[0] = None

              # ---- v2 attention: lookahead emission so the in-order PE
              # queue never head-of-line blocks on an in-flight exp, 4 DVE
              # exp pairs per slice to unload ScalarE below the PE pace, and
              # the out-projection spread one tile per op_slot.
              # ---- v3 attention: 3-key-tile score groups in [128,1536]
              # 3-bank PSUM tiles (st_bufs=2).  "a" groups: one ScalarE exp
              # over all 1536 cols (amortized access-latency init); "s"
              # groups: ScalarE takes tiles 0-1 (1024 cols), DVE takes tile 2
              # as a short 512-col 2-op chunk whose ~1.4us latency fits the
              # 2-group pv lookahead window.  pv emitted per single key tile.
              if v3 and not wide:
               pending_v3 = [None]
               nop3 = [0]

               def emit_op_tile_v3():
                   if pending_v3[0] is None:
                       return
                   isl_, ao_ = pending_v3[0]
                   t = nop3[0]
                   t_ = isl_ * 4 + t
                   cs = slice(t * P, (t + 1) * P)
                   pp = opps.tile([P, 512], f32, name="pp", tag="op")
                   nc.tensor.matmul(pp[:], ao_[:, cs], wo_sb[:], start=True, stop=True)
                   yt = yp.tile([P, C], f32, name="yt", tag="y")
                   nc.vector.tensor_scalar_mul(yt[:], pp[:], rec_all[:, t_ : t_ + 1])
                   nc.sync.dma_start(y_d[t_ * P : (t_ + 1) * P, :], yt[:])
                   nop3[0] += 1
                   if nop3[0] == 4:
                       pending_v3[0] = None
                       nop3[0] = 0

               groups = []
               _j = 0
               for kind in v3_pattern:
                   groups.append((kind, [_j, _j + 1, _j + 2]))
                   _j += 3
               groups.append(("a", [_j, _j + 1]))  # trailing pair (tiles 30,31)
               assert _j + 2 == LT

               for isl in range(NSL):
                isx = slice(isl * 512, (isl + 1) * 512)
                acc0 = None
                if "pv" not in ablate:
                    acc0 = pvps.tile([D + 1, 512], f32, name="acc0", tag="pv")
                pvq = []  # FIFO of per-group pv emitters

                def emit_group(kind, tiles):
                    w = len(tiles)
                    if "exp" in ablate or "st" in ablate:
                        if "st" not in ablate:
                            stp = stps.tile([P, 1536], f32, name="stp", tag="st")
                            for i, jt in enumerate(tiles):
                                cs = slice(i * 512, (i + 1) * 512)
                                if jt % 2 == 0:
                                    nc.tensor.matmul(
                                        stp[:, cs],
                                        kqT[:D, jt * P : (jt + 1) * P],
                                        qkT[:D, isx], start=True, stop=True)
                                else:
                                    nc.tensor.matmul(
                                        stp[:, cs],
                                        qkT[D:, jt * P : (jt + 1) * P],
                                        kqT[D:, isx], start=True, stop=True)
                        refs = [
                            (jt, e_shared[:, i * 512 : (i + 1) * 512])
                            for i, jt in enumerate(tiles)
                        ]

                        def emit_pvs():
                            if "pv" in ablate:
                                return
                            for jt, e_ap in refs:
                                nc.tensor.matmul(
                                    acc0[:], v_sb[:, jt, :], e_ap,
                                    start=(jt == 0), stop=(jt == LT - 1))

                        pvq.append(emit_pvs)
                        return
                    stp = stps.tile([P, 1536], f32, name="stp", tag="st")
                    for i, jt in enumerate(tiles):
                        cs = slice(i * 512, (i + 1) * 512)
                        if jt % 2 == 0 or score_form == "a":
                            nc.tensor.matmul(
                                stp[:, cs],
                                kqT[:D, jt * P : (jt + 1) * P],
                                qkT[:D, isx],
                                start=True, stop=True,
                            )
                        else:
                            nc.tensor.matmul(
                                stp[:, cs],
                                qkT[D:, jt * P : (jt + 1) * P],
                                kqT[D:, isx],
                                start=True, stop=True,
                            )
                    refs = []  # (key tile, e AP) for the pv stage
                    if kind == "s" and w == 3:
                        ea = expp.tile([P, 1536], bf16, name="e", tag="e")
                        nc.scalar.activation(
                            ea[:, :1024], stp[:, :1024], Exp,
                            bias=bias_t[:], scale=LN2,
                        )
                        ed = e2p.tile([P, 512], bf16, name="ed", tag="ed")
                        g = gp.tile([P, 512], i16, name="g", tag="g")
                        nc.vector._custom_dve(
                            OPA, out=g[:], in0=stp[:, 1024:],
                            s0=MAGIC, s1=A_SCALE, imm2=A_BIAS,
                        )
                        nc.vector._custom_dve(
                            OPB, out=ed[:], in0=g[:].bitcast(bf16),
                            in1=stp[:, 1024:],
                            s0=MAGIC, s1=POLY_B, imm2=POLY_C,
                        )
                        refs = [
                            (tiles[0], ea[:, :512]),
                            (tiles[1], ea[:, 512:1024]),
                            (tiles[2], ed[:]),
                        ]
                    else:
                        ea = expp.tile([P, 1536], bf16, name="e", tag="e")
                        nc.scalar.activation(
                            ea[:, : w * 512], stp[:, : w * 512], Exp,
                            bias=bias_t[:], scale=LN2,
                        )
                        refs = [
                            (jt, ea[:, i * 512 : (i + 1) * 512])
                            for i, jt in enumerate(tiles)
                        ]

                    def emit_pvs():
                        for jt, e_ap in refs:
                            nc.tensor.matmul(
                                acc0[:], v_sb[:, jt, :], e_ap,
                                start=(jt == 0), stop=(jt == LT - 1),
                            )

                    pvq.append(emit_pvs)

                for gi, (kind, tiles) in enumerate(groups):
                    emit_group(kind, tiles)
                    if gi >= 2:
                        pvq.pop(0)()
                    if gi in v3_op_groups:
                        emit_op_tile_v3()
                while pvq:
                    pvq.pop(0)()
                emit_op_tile_v3()  # 4th out-proj tile of the previous slice

                if "tail" in ablate:
                    continue
                rsum = rowp.tile([1, 512], f32, name="rsum", tag="rr")
                nc.vector.tensor_copy(rsum[:], acc0[D : D + 1, :])
                if rec_mode == "pe":
                    # transpose the row sums [1,512] -> [128,4] with 4 tiny
                    # PE transposes, reciprocal straight into rec_all: no
                    # DRAM bounce on the yt critical path
                    tp = opps.tile([P, 512], f32, name="tp", tag="op")
                    for t in range(4):
                        nc.tensor.matmul(
                            tp[:, t : t + 1],
                            rsum[:, t * P : (t + 1) * P],
                            ones1[:],
                            is_transpose=True, start=True, stop=True,
                        )
                    nc.vector._custom_dve(
                        OPR, out=rec_all[:, isl * 4 : (isl + 1) * 4],
                        in0=tp[:, :4],
                        s0=RECIP_S0, s1=RECIP_S1, imm2=RECIP_IMM2,
                    )
                else:
                    rec_row = rowp.tile([1, 512], f32, name="rec_row", tag="rr")
                    nc.vector._custom_dve(
                        OPR, out=rec_row[:], in0=rsum[:],
                        s0=RECIP_S0, s1=RECIP_S1, imm2=RECIP_IMM2,
                    )
                    dr = drsp.tile([512], f32, name="dr", tag="dr")
                    nc.sync.dma_start(dr[:], rec_row[:])
                    nc.sync.dma_start(
                        rec_all[:, isl * 4 : (isl + 1) * 4],
                        dr.rearrange("(t p) -> p t", p=P),
                    )
                ao0 = aop.tile([D, 512], bf16, name="ao0", tag="ao")
                if ao_eng == "scalar":
                    nc.scalar.activation(ao0[:], acc0[:D, :], Copy)
                else:
                    nc.vector.tensor_copy(ao0[:], acc0[:D, :])
                if "proj" not in ablate:
                    pending_v3[0] = (isl, ao0)

               while pending_v3[0] is not None:
                   emit_op_tile_v3()

              if v2 and not wide:
               pending_v2 = [None]  # (isl, ao0) of the slice awaiting outproj
               nop = [0]

               def emit_op_tile_v2():
                   if pending_v2[0] is None:
                       return
                   isl_, ao_ = pending_v2[0]
                   t = nop[0]
                   t_ = isl_ * 4 + t
                   cs = slice(t * P, (t + 1) * P)
                   pp = opps.tile([P, 512], f32, name="pp", tag="op")
                   nc.tensor.matmul(pp[:], ao_[:, cs], wo_sb[:], start=True, stop=True)
                   yt = yp.tile([P, C], f32, name="yt", tag="y")
                   nc.vector.tensor_scalar_mul(yt[:], pp[:], rec_all[:, t_ : t_ + 1])
                   nc.sync.dma_start(y_d[t_ * P : (t_ + 1) * P, :], yt[:])
                   nop[0] += 1
                   if nop[0] == 4:
                       pending_v2[0] = None
                       nop[0] = 0

               for isl in range(NSL):
                isx = slice(isl * 512, (isl + 1) * 512)
                acc0 = pvps.tile([D + 1, 512], f32, name="acc0", tag="pv")
                stq = {}
                eq = {}

                def emit_scores_v2(m):
                    jA, jB = 2 * m, 2 * m + 1
                    stp = stps.tile([P, 1024], f32, name="stp", tag="st")
                    nc.tensor.matmul(
                        stp[:, :512],
                        kqT[:D, jA * P : (jA + 1) * P],
                        qkT[:D, isx],
                        start=True,
                        stop=True,
                    )
                    nc.tensor.matmul(
                        stp[:, 512:],
                        qkT[D:, jB * P : (jB + 1) * P],
                        kqT[D:, isx],
                        start=True,
                        stop=True,
                    )
                    stq[m] = stp

                def emit_exp_v2(m):
                    stp = stq[m]
                    e = expp.tile([P, 1024], bf16, name="e", tag="e")
                    if m in dve_set:
                        g = gp.tile([P, 1024], i16, name="g", tag="g")
                        nc.vector._custom_dve(
                            OPA, out=g[:], in0=stp[:],
                            s0=MAGIC, s1=A_SCALE, imm2=A_BIAS,
                        )
                        nc.vector._custom_dve(
                            OPB, out=e[:], in0=g[:].bitcast(bf16), in1=stp[:],
                            s0=MAGIC, s1=POLY_B, imm2=POLY_C,
                        )
                    else:
                        nc.scalar.activation(
                            e[:], stp[:], Exp, bias=bias_t[:], scale=LN2
                        )
                    eq[m] = e

                def emit_pv_v2(p):
                    jA, jB = 2 * p, 2 * p + 1
                    e = eq.pop(p)
                    stq.pop(p)
                    nc.tensor.matmul(
                        acc0[:], v_sb[:, jA, :], e[:, :512],
                        start=(p == 0), stop=False,
                    )
                    nc.tensor.matmul(
                        acc0[:], v_sb[:, jB, :], e[:, 512:],
                        start=False, stop=(p == NPAIR - 1),
                    )

                for m in range(NPAIR):
                    emit_scores_v2(m)
                    emit_exp_v2(m)
                    if m >= lookahead:
                        emit_pv_v2(m - lookahead)
                    if m in op_slots:
                        emit_op_tile_v2()
                for p in range(NPAIR - lookahead, NPAIR):
                    emit_pv_v2(p)
                emit_op_tile_v2()  # 4th out-proj tile of the previous slice

                # slice tail: denominators + reciprocal + ao eviction
                rsum = rowp.tile([1, 512], f32, name="rsum", tag="rr")
                nc.vector.tensor_copy(rsum[:], acc0[D : D + 1, :])
                rec_row = rowp.tile([1, 512], f32, name="rec_row", tag="rr")
                nc.vector._custom_dve(
                    OPR, out=rec_row[:], in0=rsum[:],
                    s0=RECIP_S0, s1=RECIP_S1, imm2=RECIP_IMM2,
                )
                dr = drsp.tile([512], f32, name="dr", tag="dr")
                nc.sync.dma_start(dr[:], rec_row[:])
                nc.sync.dma_start(
                    rec_all[:, isl * 4 : (isl + 1) * 4],
                    dr.rearrange("(t p) -> p t", p=P),
                )
                ao0 = aop.tile([D, 512], bf16, name="ao0", tag="ao")
                if ao_eng == "scalar":
                    nc.scalar.activation(ao0[:], acc0[:D, :], Copy)
                else:
                    nc.vector.tensor_copy(ao0[:], acc0[:D, :])
                pending_v2[0] = (isl, ao0)

               while pending_v2[0] is not None:
                   emit_op_tile_v2()

              if not wide and not v2 and not v3:
               for isl in range(NSL):
                isx = slice(isl * 512, (isl + 1) * 512)
                if split_pv:
                    acc0 = pvps.tile([D + 1, 512], f32, name="acc0", tag="pv")
                    acc1 = pvps.tile([D + 1, 512], f32, name="acc1", tag="pv")
                else:
                    acc0 = pvps.tile([D + 1, 512], f32, name="acc0", tag="pv")
                    acc1 = None
                for m in range(NPAIR):
                    jA, jB = 2 * m, 2 * m + 1
                    stp = stps.tile([P, 1024], f32, name="stp", tag="st")
                    if "st" not in ablate:
                        nc.tensor.matmul(
                            stp[:, :512],
                            kqT[:D, jA * P : (jA + 1) * P],
                            qkT[:D, isx],
                            start=True,
                            stop=True,
                        )
                        nc.tensor.matmul(
                            stp[:, 512:],
                            qkT[D:, jB * P : (jB + 1) * P],
                            kqT[D:, isx],
                            start=True,
                            stop=True,
                        )
                    e = expp.tile([P, 1024], bf16, name="e", tag="e")
                    if "exp" not in ablate:
                        if col_split:
                            c = col_split
                            nc.scalar.activation(
                                e[:, :c], stp[:, :c], Exp, bias=bias_t[:], scale=LN2
                            )
                            g = gp.tile([P, 1024 - c], i16, name="g", tag="g")
                            nc.vector._custom_dve(
                                OPA, out=g[:], in0=stp[:, c:],
                                s0=MAGIC, s1=A_SCALE, imm2=A_BIAS,
                            )
                            nc.vector._custom_dve(
                                OPB, out=e[:, c:], in0=g[:].bitcast(bf16),
                                in1=stp[:, c:],
                                s0=MAGIC, s1=POLY_B, imm2=POLY_C,
                            )
                        elif m in dve_pairs:
                            g = gp.tile([P, 1024], i16, name="g", tag="g")
                            nc.vector._custom_dve(
                                OPA, out=g[:], in0=stp[:],
                                s0=MAGIC, s1=A_SCALE, imm2=A_BIAS,
                            )
                            nc.vector._custom_dve(
                                OPB, out=e[:], in0=g[:].bitcast(bf16), in1=stp[:],
                                s0=MAGIC, s1=POLY_B, imm2=POLY_C,
                            )
                        else:
                            nc.scalar.activation(
                                e[:], stp[:], Exp, bias=bias_t[:], scale=LN2
                            )
                    if "pv" not in ablate:
                        if split_pv:
                            nc.tensor.matmul(
                                acc0[:], v_sb[:D, jA, :], e[:D, :512],
                                start=(m == 0), stop=False,
                            )
                            nc.tensor.matmul(
                                acc1[:], v_sb[D:, jA, :], e[D:, :512],
                                start=(m == 0), stop=False,
                            )
                            nc.tensor.matmul(
                                acc0[:], v_sb[:D, jB, :], e[:D, 512:],
                                start=False, stop=(m == NPAIR - 1),
                            )
                            nc.tensor.matmul(
                                acc1[:], v_sb[D:, jB, :], e[D:, 512:],
                                start=False, stop=(m == NPAIR - 1),
                            )
                        else:
                            nc.tensor.matmul(
                                acc0[:], v_sb[:, jA, :], e[:, :512],
                                start=(m == 0), stop=False,
                            )
                            nc.tensor.matmul(
                                acc0[:], v_sb[:, jB, :], e[:, 512:],
                                start=False, stop=(m == NPAIR - 1),
                            )
                    if m == op_at and pending_outproj[0] is not None:
                        pending_outproj[0]()
                        pending_outproj[0] = None

                if "tail" in ablate:
                    continue
                # slice tail: denominators + reciprocal + ao eviction
                rsum = rowp.tile([1, 512], f32, name="rsum", tag="rr")
                nc.vector.tensor_copy(rsum[:], acc0[D : D + 1, :])
                if split_pv:
                    # DVE reads at most one PSUM input: rsum is SBUF now
                    nc.vector.tensor_add(rsum[:], rsum[:], acc1[D : D + 1, :])
                rec_row = rowp.tile([1, 512], f32, name="rec_row", tag="rr")
                nc.vector._custom_dve(
                    OPR, out=rec_row[:], in0=rsum[:],
                    s0=RECIP_S0, s1=RECIP_S1, imm2=RECIP_IMM2,
                )
                dr = drsp.tile([512], f32, name="dr", tag="dr")
                nc.sync.dma_start(dr[:], rec_row[:])
                nc.sync.dma_start(
                    rec_all[:, isl * 4 : (isl + 1) * 4],
                    dr.rearrange("(t p) -> p t", p=P),
                )
                ao0 = aop.tile([D, 512], bf16, name="ao0", tag="ao")
                if ao_eng == "scalar":
                    nc.scalar.activation(ao0[:], acc0[:D, :], Copy)
                else:
                    nc.vector.tensor_copy(ao0[:], acc0[:D, :])
                if split_pv:
                    ao1 = aop.tile([D, 512], bf16, name="ao1", tag="ao")
                    nc.vector.tensor_copy(ao1[:], acc1[:D, :])
                else:
                    ao1 = None
                if "proj" not in ablate:
                    pending_outproj[0] = emit_outproj(isl, ao0, ao1)

              if pending_outproj[0] is not None:
                  pending_outproj[0]()
                  pending_outproj[0] = None

            with rep_ctx:
                for _uu in range(u):
                    _rep_body()

    nc.compile()
    return nc


_nc_cache = {}


def _get_nc(**kw):
    key = tuple(sorted(kw.items()))
    if key not in _nc_cache:
        _nc_cache[key] = build_nc(**kw)
    return _nc_cache[key]


def make_in_maps(x, w_qkv, w_out):
    """Host-side sharding: per-head weight slices, shared transposed input.
    wq is pre-scaled by D^-1/2 * log2(e): scores arrive as t = s*log2e, so
    e^s == 2^t (ScalarE exp uses scale=ln2 to undo; the DVE path computes
    2^t directly)."""
    x = np.asarray(x, dtype=np.float32)
    w_qkv = np.asarray(w_qkv, dtype=np.float32)
    w_out = np.asarray(w_out, dtype=np.float32)
    scale = float(D) ** -0.5 * LOG2E
    xt = np.ascontiguousarray(x[0].T).astype(_BF16)  # [C, L]
    in_maps = []
    for h in range(N_CORES):
        sl = slice(h * D, (h + 1) * D)
        wq = (w_qkv[0 * C :][sl, :] * scale).T  # [C, D]
        wk = w_qkv[1 * C :][sl, :].T
        wqk = np.ascontiguousarray(np.concatenate([wq, wk], axis=1)).astype(_BF16)
        wv = np.ascontiguousarray(w_qkv[2 * C :][sl, :].T).astype(_BF16)
        wo = np.ascontiguousarray(w_out[:, sl].T).astype(_BF16)
        in_maps.append({"xt": xt, "wqk": wqk, "wv": wv, "wo": wo})
    return in_maps


def kernel(x, w_qkv, w_out, b_out):
    from concourse.bass_utils import run_bass_kernel_spmd

    nc = _get_nc()
    in_maps = make_in_maps(x, w_qkv, w_out)
    res = run_bass_kernel_spmd(nc, in_maps, list(range(N_CORES)))
    y = res.results[0]["y"].copy()
    for i in range(1, N_CORES):
        y += res.results[i]["y"]
    y += np.asarray(b_out, dtype=np.float32)
    return y[None]

